# revision 11
# baseline (speedup 1.0000x reference)
"""TRN2 Bass kernel for nn_ChaiThermoTransformer — 8-core SPMD.

pair row-sharded (96 rows/core) for the pair-bias MLP; attention row-sharded
over queries (pair_bias stays local); h all-gathered per layer; site head
computed redundantly via host-built gather matrices.
"""
import numpy as np
import ml_dtypes

import concourse.bacc as bacc
import concourse.tile as tile
import concourse.mybir as mybir
from concourse.bass_utils import run_bass_kernel_spmd

BF16 = mybir.dt.float16
F32 = mybir.dt.float32
bf = np.float16
AX = mybir.AxisListType
ALU = mybir.AluOpType
ACT = mybir.ActivationFunctionType

L, M = 768, 64
SINGLE, PAIR, DM, H, NL, DFF, SH = 384, 256, 256, 8, 4, 512, 128
HD = DM // H
MID = PAIR // 4
EPS = 1e-5
NC = 8
LW = L // NC                 # 96 own rows
POS = LW * L                 # 73728 pair positions per core
S = POS // 128               # 576: pos = p*S + s
WIN = 8                      # s per window
NW = S // WIN                # 72 windows
NT = L // 128                # 6 seq tiles
SCALE = HD ** -0.5
KC = SINGLE // 128           # 3 k-chunks for single proj

_CACHE = {}


def _build():
    nc = bacc.Bacc("TRN2", target_bir_lowering=False, debug=False,
                   num_devices=NC)

    def din(name, shape, dtype=BF16):
        return nc.dram_tensor(name, list(shape), dtype, kind="ExternalInput")

    pairb = din("pairb", [POS, PAIR])
    singleT = din("singleT", [SINGLE, L])
    singleTo = din("singleTo", [SINGLE, LW])
    w1g = din("w1g", [PAIR, MID])
    b1s = din("b1s", [128, 1], F32)
    w2s = din("w2s", [128, H])
    b2row = din("b2row", [1, 64 * H], F32)
    wp = din("wp", [SINGLE, DM])
    bp_row = din("bp_row", [1, DM])
    sng_row = din("sng_row", [1, DM], F32)
    snb_row = din("snb_row", [1, DM], F32)
    qkvw = din("qkvw", [NL, PAIR, 3 * DM])
    qkvb = din("qkvb", [NL, 3 * DM], F32)
    qkvbv = din("qkvbv", [NL, 1, DM])
    i96g = din("i96g", [NL, H, LW, LW])
    outw = din("outw", [NL, DM, DM])
    outb = din("outb", [NL, 1, DM])
    ff1w = din("ff1w", [NL, DM, DFF])
    ff1b = din("ff1b", [NL, 1, DFF])
    ff2w = din("ff2w", [NL, DFF, DM])
    ff2b = din("ff2b", [NL, 1, DM])
    gt = din("gt", [L, 65], F32)
    wa = din("wa", [DM, SH])
    wb = din("wb", [DM, SH])
    bc_row = din("bc_row", [1, SH], F32)
    r2 = din("r2", [2, M], F32)
    wc_row = din("wc_row", [1, SH], F32)
    w2site = din("w2site", [SH, MID])
    b2site = din("b2site", [1, MID])
    w3pad = din("w3pad", [SH, 20])
    b3row = din("b3row", [1, 20])
    dmat = din("dmat", [M, 20], F32)

    out_t = nc.dram_tensor("out", [M, 1], F32, kind="ExternalOutput")

    with tile.TileContext(nc) as tc:
        import contextlib
        with contextlib.ExitStack() as ctx:
            P = lambda name, bufs, **kw: ctx.enter_context(
                tc.tile_pool(name=name, bufs=bufs, **kw))
            singles = P("singles", 1)
            wpool = P("wpool", 2)
            a_nat = P("a_nat", 3)
            a_xt = P("a_xt", 3)
            a_st = P("a_st", 2)
            a_mid = P("a_mid", 3)
            big = P("big", 2)
            b_lay = P("b_lay", 1)
            b_at = P("b_at", 2)
            b_sm = P("b_sm", 2)
            ps_lg = P("ps_lg", 1, space="PSUM")     # [96,768] 2-bank slot
            ps_mid = P("ps_mid", 2, space="PSUM")   # pmid [128,512]
            ps_pb2 = P("ps_pb2", 2, space="PSUM")   # pb2 [128,512]
            ps_ms = P("ps_ms", 2, space="PSUM")     # misc 1-bank
            dram = P("dram", 2, space="DRAM")
            dram_sh = P("dram_sh", 2, space="DRAM")

            sy, ve, sc, gp, te = nc.sync, nc.vector, nc.scalar, nc.gpsimd, nc.tensor

            # ---------- persistent state ----------
            h_full = singles.tile([128, NT, DM], F32)
            bias_sb = singles.tile([LW, H, L], BF16)
            w1g_sb = singles.tile([128, 2, MID], BF16)
            sy.dma_start(w1g_sb, w1g[:, :].rearrange("(h p) m -> p h m", p=128))
            w2s_sb = singles.tile([128, H], BF16)
            sy.dma_start(w2s_sb, w2s[:, :])
            b1s_sb = singles.tile([128, 1], F32)
            sy.dma_start(b1s_sb, b1s[:, :])
            b2bc = singles.tile([128, 64 * H], F32)
            gp.dma_start(b2bc, b2row[:, :].to_broadcast([128, 64 * H]))
            ones_sb = singles.tile([1, 128], BF16)
            ve.memset(ones_sb, 1.0)
            sng_bc = singles.tile([128, DM], F32)
            gp.dma_start(sng_bc, sng_row[:, :].to_broadcast([128, DM]))
            snb_bc = singles.tile([128, DM], F32)
            gp.dma_start(snb_bc, snb_row[:, :].to_broadcast([128, DM]))
            biasdram = dram.tile([H, POS], BF16)

            def rank1(psum, brow_sb, Mp, N, stop=True):
                te.matmul(psum, ones_sb[:, 0:Mp], brow_sb[:, 0:N],
                          start=False, stop=stop)

            # LN stats+normalize over free dim D for [Pn, n, D] tiles.
            # rsqrt: mode 'nr' (DVE Newton, var~1) or 'act' (exp(-.5 ln)).
            def layernorm(dst, src, Pn, n, D, stp, mode, sqdt=F32):
                sumx = stp.tile([Pn, n], F32, tag="lsx")
                ve.reduce_sum(sumx, src, axis=AX.X)
                sq = stp.tile([Pn, n, D], sqdt, tag="lsq")
                ve.tensor_tensor(out=sq, in0=src, in1=src, op=ALU.mult)
                sumsq = stp.tile([Pn, n], F32, tag="lss")
                ve.reduce_sum(sumsq, sq, axis=AX.X)
                negmu = stp.tile([Pn, n], F32, tag="lmu")
                ve.tensor_scalar(out=negmu, in0=sumx, scalar1=-1.0 / D,
                                 scalar2=None, op0=ALU.mult)
                mu2 = stp.tile([Pn, n], F32, tag="lm2")
                ve.tensor_tensor(out=mu2, in0=negmu, in1=negmu, op=ALU.mult)
                msq = stp.tile([Pn, n], F32, tag="lms")
                ve.tensor_scalar(out=msq, in0=sumsq, scalar1=1.0 / D,
                                 scalar2=EPS, op0=ALU.mult, op1=ALU.add)
                var = stp.tile([Pn, n], F32, tag="lvv")
                ve.tensor_tensor(out=var, in0=msq, in1=mu2, op=ALU.subtract)
                r = stp.tile([Pn, n], F32, tag="lrr")
                if mode == "nr":
                    y = stp.tile([Pn, n], F32, tag="lnr")
                    ve.tensor_scalar(out=y, in0=var, scalar1=-0.55,
                                     scalar2=1.62, op0=ALU.mult, op1=ALU.add)
                    for _ in range(2):
                        t = stp.tile([Pn, n], F32, tag="lnt")
                        ve.tensor_tensor(out=t, in0=y, in1=y, op=ALU.mult)
                        ve.tensor_tensor(out=t, in0=t, in1=var, op=ALU.mult)
                        ve.tensor_scalar(out=t, in0=t, scalar1=-0.5,
                                         scalar2=1.5, op0=ALU.mult, op1=ALU.add)
                        ve.tensor_tensor(out=y, in0=y, in1=t, op=ALU.mult)
                    r = y
                else:
                    lnv = stp.tile([Pn, n], F32, tag="llv")
                    sc.activation(lnv, var, ACT.Ln)
                    sc.activation(r, lnv, ACT.Exp, scale=-0.5)
                for j in range(n):
                    ve.tensor_scalar(out=dst[:, j], in0=src[:, j],
                                     scalar1=negmu[:, j:j + 1],
                                     scalar2=r[:, j:j + 1],
                                     op0=ALU.add, op1=ALU.mult)

            # ---------- stage 0: h0 ----------
            st_sb = big.tile([128, KC, L], BF16, tag="big")
            sy.dma_start(st_sb, singleT[:, :].rearrange("(k p) f -> p k f", p=128))
            sto_sb = singles.tile([128, KC, LW], BF16)
            sy.dma_start(sto_sb, singleTo[:, :].rearrange("(k p) f -> p k f", p=128))
            wp_sb = singles.tile([128, KC, DM], BF16)
            sy.dma_start(wp_sb, wp[:, :].rearrange("(k p) m -> p k m", p=128))
            bp_sb = singles.tile([1, DM], BF16)
            sy.dma_start(bp_sb, bp_row[:, :])

            h0raw = big.tile([128, NT, DM], F32, tag="big")
            for t in range(NT):
                ph = ps_ms.tile([128, DM], F32, tag="ms")
                for k in range(KC):
                    te.matmul(ph, st_sb[:, k, t * 128:(t + 1) * 128],
                              wp_sb[:, k], start=(k == 0), stop=False)
                rank1(ph, bp_sb, 128, DM)
                sc.copy(h0raw[:, t], ph)
            pho = ps_ms.tile([LW, DM], F32, tag="ms")
            for k in range(KC):
                te.matmul(pho, sto_sb[:, k], wp_sb[:, k],
                          start=(k == 0), stop=False)
            rank1(pho, bp_sb, LW, DM)
            h0rawo = singles.tile([LW, 1, DM], F32)
            sc.copy(h0rawo[:, 0], pho)

            hn0 = big.tile([128, NT, DM], F32, tag="big")
            layernorm(hn0, h0raw, 128, NT, DM, a_st, "act", sqdt=BF16)
            for t in range(NT):
                ve.tensor_tensor(out=h_full[:, t], in0=hn0[:, t],
                                 in1=sng_bc, op=ALU.mult)
                ve.tensor_tensor(out=h_full[:, t], in0=h_full[:, t],
                                 in1=snb_bc, op=ALU.add)
            hn0o = singles.tile([LW, 1, DM], F32)
            layernorm(hn0o, h0rawo, LW, 1, DM, a_st, "act", sqdt=BF16)
            h_own = singles.tile([LW, DM], F32)
            ve.tensor_tensor(out=h_own, in0=hn0o[:, 0], in1=sng_bc[0:LW],
                             op=ALU.mult)
            ve.tensor_tensor(out=h_own, in0=h_own, in1=snb_bc[0:LW],
                             op=ALU.add)

            # ---------- phase A: pair bias ----------
            pv = pairb[:, :].rearrange("(p s) c -> p s c", p=128)
            bias_acc = singles.tile([128, S, H], BF16)
            pb2 = None
            for w in range(NW):
                nat = a_nat.tile([128, WIN, PAIR], BF16, tag="nat")
                sy.dma_start(nat, pv[:, w * WIN:(w + 1) * WIN, :])
                xn = a_nat.tile([128, WIN, PAIR], BF16, tag="xn")
                layernorm(xn, nat, 128, WIN, PAIR, a_st, "nr", sqdt=BF16)
                xt = a_xt.tile([128, WIN * 2, 128], BF16, tag="xt")
                sy.dma_start(xt, xn, transpose=True)
                xtr = xt[:, :, :].rearrange("p (s h) f -> p h s f", h=2)

                pmid = ps_mid.tile([128, 512], F32, tag="pmid")
                for g in range(2):           # 4 s per group
                    for half in range(2):
                        te.matmul(pmid[64 * g:64 * g + 64, :],
                                  w1g_sb[:, half],
                                  xtr[:, half, 4 * g:4 * g + 4, :],
                                  start=(half == 0), stop=(half == 1),
                                  tile_position=(0, 64 * g))
                mid = a_mid.tile([128, 512], BF16, tag="mid")
                sc.activation(mid, pmid, ACT.Gelu, bias=b1s_sb)

                if w % 8 == 0:
                    pb2 = ps_pb2.tile([128, 64 * H], F32, tag="pb2")
                for sl in range(WIN):
                    g, i = sl // 4, sl % 4
                    s_glob = w * WIN + sl
                    so = (s_glob % 64) * H
                    te.matmul(pb2[:, so:so + H],
                              mid[64 * g:64 * g + 64, 128 * i:128 * i + 128],
                              w2s_sb[64 * g:64 * g + 64, :],
                              start=True, stop=True)
                if w % 8 == 7:
                    ve.tensor_tensor(out=pb2, in0=pb2, in1=b2bc, op=ALU.add)
                    blk = w // 8
                    sc.activation(bias_acc[:, blk * 64:(blk + 1) * 64, :],
                                  pb2, ACT.Tanh)

            for h in range(H):
                for pp in range(2):
                    sy.dma_start(
                        biasdram[h, pp * 64 * S:(pp + 1) * 64 * S]
                        .rearrange("(p s) -> p s", p=64),
                        bias_acc[64 * pp:64 * pp + 64, :, h])
            for h in range(H):
                sy.dma_start(bias_sb[:, h, :],
                             biasdram[h, :].rearrange("(i j) -> i j", i=LW))

            # ---------- phase B: transformer layers ----------
            ho_cur = h_own
            for l in range(NL):
                qw = wpool.tile([128, 2, 3 * DM], BF16, tag="qw")
                sy.dma_start(qw, qkvw[l, :, :].rearrange("(k p) m -> p k m", p=128))
                qb = wpool.tile([128, 6], F32, tag="qb")
                sy.dma_start(qb, qkvb[l, :].rearrange("(c p) -> p c", p=128))
                qbv = wpool.tile([1, DM], BF16, tag="qbv")
                sy.dma_start(qbv, qkvbv[l, :, :])
                ow = wpool.tile([128, 2, DM], BF16, tag="ow")
                sy.dma_start(ow, outw[l, :, :].rearrange("(k p) m -> p k m", p=128))
                ob = wpool.tile([1, DM], BF16, tag="ob")
                sy.dma_start(ob, outb[l, :, :])
                f1w = wpool.tile([128, 2, DFF], BF16, tag="f1w")
                sy.dma_start(f1w, ff1w[l, :, :].rearrange("(k p) m -> p k m", p=128))
                f1b = wpool.tile([1, DFF], BF16, tag="f1b")
                sy.dma_start(f1b, ff1b[l, :, :])
                f2w = wpool.tile([128, 4, DM], BF16, tag="f2w")
                sy.dma_start(f2w, ff2w[l, :, :].rearrange("(k p) m -> p k m", p=128))
                f2b = wpool.tile([1, DM], BF16, tag="f2b")
                sy.dma_start(f2b, ff2b[l, :, :])
                i96_sb = wpool.tile([LW, H, LW], BF16, tag="i96")
                sy.dma_start(i96_sb, i96g[l, :, :, :].rearrange("h i j -> i h j"))

                # LN1 over full h + own rows
                xn1 = b_lay.tile([128, NT, DM], BF16, tag="xn1")
                layernorm(xn1, h_full, 128, NT, DM, a_st, "act", sqdt=BF16)
                xn1T = b_lay.tile([128, NT * 2, 128], BF16, tag="xn1T")
                for t in range(NT):
                    sy.dma_start(xn1T[:, 2 * t:2 * t + 2, :], xn1[:, t, :],
                                 transpose=True)
                xn1o = b_sm.tile([LW, 1, DM], BF16, tag="xn1o")
                layernorm(xn1o, ho_cur[:, :].unsqueeze(1),
                          LW, 1, DM, a_st, "act", sqdt=BF16)
                xn1oT = b_sm.tile([128, 2, LW], BF16, tag="xn1oT")
                sy.dma_start(xn1oT, xn1o[:, 0, :], transpose=True)

                # kT (qkv cols 256:512) as [128, 2, L]
                kT = b_lay.tile([128, 2, L], BF16, tag="kT")
                for mc in (2, 3):
                    for nh in range(2):
                        pq = ps_ms.tile([128, 384], F32, tag="ms")
                        xr = xn1T[:, :, :].rearrange(
                            "p (t h) f -> p h t f", h=2)
                        for k in range(2):
                            te.matmul(pq, qw[:, k, mc * 128:(mc + 1) * 128],
                                      xr[:, k, 3 * nh:3 * nh + 3, :],
                                      start=(k == 0), stop=(k == 1))
                        sc.activation(kT[:, mc - 2, 384 * nh:384 * (nh + 1)],
                                      pq, ACT.Identity, bias=qb[:, mc:mc + 1])
                # qT own cols, scaled
                qT = b_sm.tile([128, 2, LW], BF16, tag="qT")
                for mc in (0, 1):
                    pq = ps_ms.tile([128, LW], F32, tag="ms")
                    for k in range(2):
                        te.matmul(pq, qw[:, k, mc * 128:(mc + 1) * 128],
                                  xn1oT[:, k, :], start=(k == 0), stop=(k == 1))
                    sc.activation(qT[:, mc, :], pq, ACT.Identity,
                                  bias=qb[:, mc:mc + 1], scale=SCALE)
                # v natural [128, NT, DM]
                vn = b_lay.tile([128, NT, DM], BF16, tag="vn")
                for t in range(NT):
                    pvn = ps_ms.tile([128, DM], F32, tag="ms")
                    for k in range(2):
                        te.matmul(pvn, xn1T[:, 2 * t + k, :],
                                  qw[:, k, 512:768], start=(k == 0), stop=False)
                    rank1(pvn, qbv, 128, DM)
                    sc.copy(vn[:, t], pvn)

                # attention
                po = ps_ms.tile([LW, DM], F32, tag="ms")
                for h in range(H):
                    mc, so = h // 4, 32 * (h % 4)
                    tp = (so, 0) if so else None
                    plg = ps_lg.tile([LW, L], F32, tag="plg")
                    for nh in range(2):
                        te.matmul(plg[:, 384 * nh:384 * (nh + 1)],
                                  qT[so:so + 32, mc, :],
                                  kT[so:so + 32, mc, 384 * nh:384 * (nh + 1)],
                                  start=True, stop=False, tile_position=tp)
                        te.matmul(plg[:, 384 * nh:384 * (nh + 1)],
                                  i96_sb[:, h, :],
                                  bias_sb[:, h, 384 * nh:384 * (nh + 1)],
                                  start=False, stop=True)
                    ex = b_at.tile([LW, L], BF16, tag="ex")
                    rs = b_sm.tile([LW, 1], F32, tag="rs")
                    sc.activation(ex, plg, ACT.Exp, accum_out=rs)
                    rv = b_sm.tile([LW, 1], F32, tag="rv")
                    ve.reciprocal(rv, rs)
                    exn = b_at.tile([LW, L], BF16, tag="exn")
                    ve.tensor_scalar(out=exn, in0=ex, scalar1=rv,
                                     scalar2=None, op0=ALU.mult)
                    attnT = b_at.tile([128, NT, LW], BF16, tag="attnT")
                    sy.dma_start(attnT, exn, transpose=True)
                    for t in range(NT):
                        te.matmul(po[:, 32 * h:32 * h + 32],
                                  attnT[:, t, :], vn[:, t, 32 * h:32 * h + 32],
                                  start=(t == 0), stop=(t == NT - 1))
                o_sb = b_sm.tile([LW, DM], BF16, tag="osb")
                sc.copy(o_sb, po)
                oT = b_sm.tile([128, 2, LW], BF16, tag="oT")
                sy.dma_start(oT, o_sb, transpose=True)
                phn = ps_ms.tile([LW, DM], F32, tag="ms")
                for k in range(2):
                    te.matmul(phn, oT[:, k, :], ow[:, k],
                              start=(k == 0), stop=False)
                rank1(phn, ob, LW, DM)
                h2o = b_sm.tile([LW, DM], F32, tag="h2o")
                ve.tensor_tensor(out=h2o, in0=ho_cur, in1=phn, op=ALU.add)

                # ffn (own rows)
                xn2 = b_sm.tile([LW, 1, DM], BF16, tag="xn2")
                layernorm(xn2, h2o[:, :].unsqueeze(1),
                          LW, 1, DM, a_st, "act", sqdt=BF16)
                xn2T = b_sm.tile([128, 2, LW], BF16, tag="xn2T")
                sy.dma_start(xn2T, xn2[:, 0, :], transpose=True)
                pf1 = ps_ms.tile([LW, DFF], F32, tag="ms")
                for k in range(2):
                    te.matmul(pf1, xn2T[:, k, :], f1w[:, k],
                              start=(k == 0), stop=False)
                rank1(pf1, f1b, LW, DFF)
                g1 = b_sm.tile([LW, DFF], BF16, tag="g1")
                sc.activation(g1, pf1, ACT.Gelu)
                g1T = b_sm.tile([128, 4, LW], BF16, tag="g1T")
                sy.dma_start(g1T, g1, transpose=True)
                pf2 = ps_ms.tile([LW, DM], F32, tag="ms")
                for k in range(4):
                    te.matmul(pf2, g1T[:, k, :], f2w[:, k],
                              start=(k == 0), stop=False)
                rank1(pf2, f2b, LW, DM)
                h3o = b_sm.tile([LW, DM], F32, tag="h3o")
                ve.tensor_tensor(out=h3o, in0=h2o, in1=pf2, op=ALU.add)
                ho_cur = h3o

                # all-gather h
                gin = dram.tile([LW, DM], F32, tag="gin")
                sy.dma_start(gin, h3o)
                gout = dram_sh.tile([L, DM], F32, tag="gout",
                                    addr_space="Shared")
                gp.collective_compute(
                    "AllGather", ALU.bypass,
                    replica_groups=[list(range(NC))],
                    ins=[gin[:, :]], outs=[gout[:, :]])
                sy.dma_start(h_full,
                             gout[:, :].rearrange("(t p) c -> p t c", p=128))

            # ---------- phase C: site head ----------
            xnf = big.tile([128, NT, DM], F32, tag="big")
            layernorm(xnf, h_full, 128, NT, DM, a_st, "act", sqdt=BF16)
            gt_sb = singles.tile([128, NT, 65], F32)
            sy.dma_start(gt_sb, gt[:, :].rearrange("(t p) c -> p t c", p=128))
            pg = ps_lg.tile([65, DM], F32, tag="plg")
            for t in range(NT):
                te.matmul(pg, gt_sb[:, t, :], xnf[:, t, :],
                          start=(t == 0), stop=(t == NT - 1))
            g80 = b_sm.tile([80, DM], BF16, tag="g80")
            ve.memset(g80, 0.0)
            sc.copy(g80[0:65, :], pg)
            gT = b_sm.tile([128, 2, 80], BF16, tag="gT")
            sy.dma_start(gT, g80, transpose=True)

            wa_sb = singles.tile([128, 2, SH], BF16)
            sy.dma_start(wa_sb, wa[:, :].rearrange("(k p) m -> p k m", p=128))
            wb_sb = singles.tile([128, 2, SH], BF16)
            sy.dma_start(wb_sb, wb[:, :].rearrange("(k p) m -> p k m", p=128))
            bc_sb = singles.tile([1, SH], F32)
            sy.dma_start(bc_sb, bc_row[:, :])
            r2_sb = singles.tile([2, M], F32)
            sy.dma_start(r2_sb, r2[:, :])
            w2s_site = singles.tile([128, MID], BF16)
            sy.dma_start(w2s_site, w2site[:, :])
            b2s_sb = singles.tile([1, MID], BF16)
            sy.dma_start(b2s_sb, b2site[:, :])
            w3_sb = singles.tile([128, 20], BF16)
            sy.dma_start(w3_sb, w3pad[:, :])
            b3_sb = singles.tile([1, 20], BF16)
            sy.dma_start(b3_sb, b3row[:, :])
            dm_sb = singles.tile([M, 20], F32)
            sy.dma_start(dm_sb, dmat[:, :])

            # hg @ wb' -> [1, SH]
            pvg = ps_ms.tile([1, SH], F32, tag="ms")
            for k in range(2):
                te.matmul(pvg, gT[:, k, 64:65], wb_sb[:, k],
                          start=(k == 0), stop=(k == 1))
            q2 = b_sm.tile([2, SH], F32, tag="q2")
            ve.tensor_tensor(out=q2[0:1, :], in0=pvg, in1=bc_sb, op=ALU.add)
            sy.dma_start(q2[1:2, :], wc_row[:, :])

            pz1 = ps_ms.tile([M, SH], F32, tag="ms")
            for k in range(2):
                te.matmul(pz1, gT[:, k, 0:64], wa_sb[:, k],
                          start=(k == 0), stop=False)
            te.matmul(pz1, r2_sb, q2, start=False, stop=True)
            z1 = b_sm.tile([M, SH], BF16, tag="z1")
            sc.activation(z1, pz1, ACT.Gelu)
            z1T = b_sm.tile([128, M], BF16, tag="z1T")
            sy.dma_start(z1T, z1, transpose=True)
            pz2 = ps_ms.tile([M, MID], F32, tag="ms")
            te.matmul(pz2, z1T, w2s_site, start=True, stop=False)
            rank1(pz2, b2s_sb, M, MID)
            z2 = b_sm.tile([M, 128], BF16, tag="z2")
            ve.memset(z2, 0.0)
            sc.activation(z2[:, 0:MID], pz2, ACT.Gelu)
            z2T = b_sm.tile([128, M], BF16, tag="z2T")
            sy.dma_start(z2T, z2, transpose=True)
            pz3 = ps_ms.tile([M, 20], F32, tag="ms")
            te.matmul(pz3, z2T, w3_sb, start=True, stop=False)
            rank1(pz3, b3_sb, M, 20)
            scr = b_sm.tile([M, 20], F32, tag="scr")
            res = b_sm.tile([M, 1], F32, tag="res")
            # TTR custom-DVE op is broken in this container: mult+reduce
            ve.tensor_tensor(out=scr, in0=pz3, in1=dm_sb, op=ALU.mult)
            ve.reduce_sum(res, scr, axis=AX.X)
            sy.dma_start(out_t[:, :], res)

    nc.finalize()
    return nc


def _host_prep(single, pair, positions, wt_indices, mut_indices, params):
    p = params
    tb = lambda a: np.ascontiguousarray(np.asarray(a, np.float32)).astype(bf)
    tf = lambda a: np.ascontiguousarray(np.asarray(a, np.float32))

    com = {}
    com["singleT"] = tb(np.asarray(single, np.float32).T)
    w1 = np.asarray(p["pb_w1"], np.float32)
    g1 = np.asarray(p["pb_ln_g"], np.float32)
    b1 = np.asarray(p["pb_ln_b"], np.float32)
    com["w1g"] = tb(g1[:, None] * w1)
    b1p = (b1 @ w1 + np.asarray(p["pb_b1"], np.float32))
    com["b1s"] = tf(np.concatenate([b1p, b1p])[:, None])
    w2 = np.asarray(p["pb_w2"], np.float32)
    com["w2s"] = tb(np.concatenate([w2, w2], 0))
    b2 = np.asarray(p["pb_b2"], np.float32)
    com["b2row"] = tf(np.tile(b2, 64)[None, :])
    com["wp"] = tb(p["single_proj_w"])
    com["bp_row"] = tb(np.asarray(p["single_proj_b"], np.float32)[None, :])
    com["sng_row"] = tf(np.asarray(p["single_norm_g"], np.float32)[None, :])
    com["snb_row"] = tf(np.asarray(p["single_norm_b"], np.float32)[None, :])

    qkvw, qkvb, qkvbv, i96, outw, outb = [], [], [], [], [], []
    f1w, f1b, f2w, f2b = [], [], [], []
    gl = np.asarray(p["gate_logits"], np.float32).reshape(NL, H)
    eye = np.eye(LW, dtype=np.float32)
    for l in range(NL):
        lp = p["layers"][l]
        g = np.asarray(lp["ln1_g"], np.float32)
        b = np.asarray(lp["ln1_b"], np.float32)
        qw = np.asarray(lp["qkv_w"], np.float32)
        qkvw.append(g[:, None] * qw)
        qb = b @ qw + np.asarray(lp["qkv_b"], np.float32)
        qkvb.append(qb)
        qkvbv.append(qb[None, 512:768])
        gate = 1.0 / (1.0 + np.exp(-gl[l]))
        i96.append(np.stack([0.1 * gate[h] * eye for h in range(H)]))
        outw.append(np.asarray(lp["out_w"], np.float32))
        outb.append(np.asarray(lp["out_b"], np.float32)[None, :])
        g2 = np.asarray(lp["ln2_g"], np.float32)
        bb2 = np.asarray(lp["ln2_b"], np.float32)
        fw1 = np.asarray(lp["ff1_w"], np.float32)
        f1w.append(g2[:, None] * fw1)
        f1b.append((bb2 @ fw1 + np.asarray(lp["ff1_b"], np.float32))[None, :])
        f2w.append(np.asarray(lp["ff2_w"], np.float32))
        f2b.append(np.asarray(lp["ff2_b"], np.float32)[None, :])
    com["qkvw"] = tb(np.stack(qkvw))
    com["qkvb"] = tf(np.stack(qkvb))
    com["qkvbv"] = tb(np.stack(qkvbv))
    com["i96g"] = tb(np.stack(i96))
    com["outw"] = tb(np.stack(outw))
    com["outb"] = tb(np.stack(outb))
    com["ff1w"] = tb(np.stack(f1w))
    com["ff1b"] = tb(np.stack(f1b))
    com["ff2w"] = tb(np.stack(f2w))
    com["ff2b"] = tb(np.stack(f2b))

    pos = np.asarray(positions, np.int64)
    gtm = np.zeros((L, 65), np.float32)
    gtm[pos, np.arange(M)] = 1.0
    gtm[:, 64] = 1.0 / L
    com["gt"] = gtm
    gf = np.asarray(p["final_norm_g"], np.float32)
    bf_ = np.asarray(p["final_norm_b"], np.float32)
    w1s = np.asarray(p["site_w1"], np.float32)
    wa_, wb_, wc_ = w1s[:DM], w1s[DM:2 * DM], w1s[2 * DM]
    com["wa"] = tb(gf[:, None] * wa_)
    com["wb"] = tb(gf[:, None] * wb_)
    com["bc_row"] = tf((bf_ @ wa_ + bf_ @ wb_ +
                        np.asarray(p["site_b1"], np.float32))[None, :])
    com["wc_row"] = tf(wc_[None, :])
    com["r2"] = np.stack([np.ones(M, np.float32),
                          pos.astype(np.float32) / L]).astype(np.float32)
    com["w2site"] = tb(p["site_w2"])
    com["b2site"] = tb(np.asarray(p["site_b2"], np.float32)[None, :])
    w3 = np.asarray(p["site_w3"], np.float32)
    com["w3pad"] = tb(np.concatenate([w3, np.zeros((SH - MID, 20), np.float32)]))
    com["b3row"] = tb(np.asarray(p["site_b3"], np.float32)[None, :])
    wt = np.asarray(wt_indices, np.int64)
    mu = np.asarray(mut_indices, np.int64)
    dmt = np.zeros((M, 20), np.float32)
    dmt[np.arange(M), mu] += 1.0
    dmt[np.arange(M), wt] -= 1.0
    com["dmat"] = dmt

    pairf = np.asarray(pair, np.float32)
    st = np.asarray(single, np.float32).T
    in_maps = []
    for c in range(NC):
        m = dict(com)
        m["pairb"] = pairf[c * LW:(c + 1) * LW].reshape(POS, PAIR).astype(bf)
        m["singleTo"] = np.ascontiguousarray(
            st[:, c * LW:(c + 1) * LW]).astype(bf)
        in_maps.append(m)
    return in_maps


LAST_EXEC_NS = [None]


def kernel(single, pair, positions, wt_indices, mut_indices, params):
    import time
    if "nc" not in _CACHE:
        _CACHE["nc"] = _build()
    nc = _CACHE["nc"]
    in_maps = _host_prep(single, pair, positions, wt_indices, mut_indices,
                         params)
    t0 = time.perf_counter()
    res = run_bass_kernel_spmd(nc, in_maps, list(range(NC)))
    LAST_EXEC_NS[0] = int((time.perf_counter() - t0) * 1e9)
    return np.asarray(res.results[0]["out"], np.float32).reshape(M)
